# revision 9
# baseline (speedup 1.0000x reference)
"""Trainium2 Bass kernel for nn_FLossNoSoftMax (topk_masking).

Computes  -sum_b mean_v[(1-mask)*log(1-x)]  where mask marks the top-c
entries per row of x [2048, 50257] f32.

Math: per row  loss_b = (S_b - T_b)/V  with
  S_b = sum_v log(1-x[b,v])
  T_b = sum over the c largest values m of log(1-m)   (multiset, tie-exact)
result = -sum_b loss_b.

Device kernel (per core, 256 rows = 2 blocks of 128 partitions): stream
row-chunks via SWDGE (gpsimd) loads — the gpsimd ring sustains the
contended HBM rate (~365-390 GB/s with all 8 cores streaming)
gaplessly.  Scalar engine computes Ln(1-x) (fused per-partition
accumulation for the head chunks); vector engine computes per-chunk
top-8 values (InstMax); chunk top-8s merge with InstMax cascades giving
the exact global top-8 multiset per row, whose first c entries yield
the T terms.

Critical-path design (v2 — tail/teardown rework):
- Measured-time anatomy: the NEFF wrapper's epilogue flood (each engine
  serially clearing its ~51-semaphore slice of S[3..255]) is ~6.9us of
  FIXED cost that starts once the slowest engine reaches the wrapper
  barrier, and the reported exec window runs from the first "useful"
  instruction to the very end of that flood.  So every ns shaved off
  the post-stream critical chain (tail compute -> store -> teardown
  arrival) is a ns off the metric.
- The opening loads fan out over THREE descgen streams (sync + scalar
  HWDGE rings, gpsimd SWDGE) to build ring depth fast; the bulk stays
  on gpsimd.  The ramp loads are emitted before any Scalar compute so
  descgen is never head-of-line blocked behind a data-gated ACT.
- Bass's __init__ preamble (4 const-tile memsets + an all-engine
  barrier) is patched out during construction: nothing in this kernel
  reads the const tiles, and the barrier delayed the first load descgen
  by ~0.7us.
- Block 1's stream ends with six shrinking chunks (832..625) sized so
  Scalar/DVE keep pace with the stream's tail; the per-chunk row-sums
  of the five middle tail chunks move to the otherwise-idle Pool
  (gpsimd) engine as tensor_reduce, so Scalar carries no accumulator
  reads there and its post-last-byte chain is just ACT(625)+read+lnm.
- The top-c log terms are NOT accumulated on Scalar: lnm values land in
  fin columns and the final ones.T @ fin matmul (idle PE engine) sums
  them; the host applies per-column signs.  This kills the last
  accumulator-read on the critical chain and consolidates the output
  into one partition so the store is a single 512B line-rate write.
- The final PSUM->SBUF copy and the output store descgen run on the
  SCALAR engine (free right after lnm; the store rides qActDynamicHW
  behind only the two early ramp loads), avoiding a cross-engine hop
  before descgen.
- Teardown (see _fast_teardown): no all-engine barriers of our own;
  GpSimd alone waits for everything (incl. the final store receipt),
  resets DGE state and releases DVE; Sync/Scalar/PE run ahead into the
  NEFF wrapper epilogue.

Output: per-column row-sums in out[0, :C]; host applies signs and the
final -sum/V in float64.
Sharding: data-parallel over the batch dim, 256 rows per core on 8 cores.
"""

import sys

sys.path.insert(0, "/opt/trn_rl_repo")

import numpy as np

from concourse import bacc, bass, mybir, tile
from concourse.bass_utils import run_bass_kernel_spmd
from concourse.vector_clock import ScopedClock


def _ensure_axon_hooks():
    """The agent image lacks antenv.axon_hooks; run_bass_kernel_spmd imports
    it when tracing is requested (e.g. BASS_TRACE=1). Provide the module and
    wire the ctypes NTFF hook so tracing works instead of crashing."""
    try:
        import antenv.axon_hooks  # noqa: F401

        return
    except ImportError:
        pass
    import types

    try:
        import antenv
    except ImportError:
        return
    mod = types.ModuleType("antenv.axon_hooks")
    store = {"h": None}
    mod.set_axon_ntff_profile_hook = lambda h: store.__setitem__("h", h)
    mod.get_axon_ntff_profile_hook = lambda: store.get("h")
    sys.modules["antenv.axon_hooks"] = mod
    antenv.axon_hooks = mod
    try:
        from trn_agent_boot.trn_boot import _ntff_profile_via_ctypes

        mod.set_axon_ntff_profile_hook(
            _ntff_profile_via_ctypes("/opt/axon/libaxon_pjrt.so")
        )
        from concourse import bass_utils as _bu

        _bu.upload_artifacts = lambda d: "local://" + d
    except Exception:
        pass


_ensure_axon_hooks()


def _fast_teardown(self, tick_clock, wait_clock):
    # Replaces Tile's stock drain + 2x all-engine-barrier tail.  The NEFF
    # wrapper's epilogue (an all-engine barrier, then each engine serially
    # clearing a ~50-semaphore slice of S[3..255], then a final barrier)
    # starts once the slowest engine exits our code — so the teardown here
    # avoids gating fast engines on slow ones.  GpSimd alone waits for all
    # outstanding body work (including the final store's HBM write
    # receipt), resets DGE state + clears the tile sems, then releases
    # Vector; Sync/Scalar/PE proceed straight to the wrapper barrier.
    nc = self.nc
    gp = nc.gpsimd.engine
    # Cheap pipeline drains on everything but GpSimd (a GpSimd drain is a
    # dge_drain, which is expensive).
    for eng_type, eng in nc.engines.items():
        if eng_type == gp:
            continue
        d = mybir.InstDrain(
            name=nc.get_next_instruction_name(), ins=[], outs=[],
            bass_is_fusable=False,
        )
        d.engine = eng_type
        eng.add_instruction(d)

    popped = nc._tile_sem_poison_stack.pop()
    assert popped is self._sem_poison

    rel = nc.alloc_semaphore("teardown_release")

    # GpSimd: wait for all outstanding body work (the attached sem waits
    # include every DMA's completion), then reset + clear the tile sems.
    sems = list(self.sems.allocated().values())
    sem_nums = [
        s.num if isinstance(s, bass.SemaphoreHandle) else s for s in sems
    ]
    first = True
    for sem_range in bass.compact_to_ranges(sem_nums):
        assert nc._state.free_isdisjoint(sem_range)
        r = nc.gpsimd.dma_reset(sem_range)
        if first:
            wait_clock.add_sem_waits(
                r.ins, ScopedClock({None: tick_clock.global_clock})
            )
            first = False
        nc.gpsimd.sem_clear(sem_range)
    nc._state.prepend_free_semaphores(sem_nums)
    for poison_set in nc._tile_sem_poison_stack:
        poison_set.update(sem_nums)

    nc.gpsimd.sem_inc(rel, 1)
    nc.vector.wait_ge(rel, 1)
    # Explicit clear so a second NEFF execution starts from 0 even if the
    # wrapper flood's coverage of `rel` shifts.
    nc.vector.sem_clear(range(rel.num, rel.num + 1))


tile.TileContext._drain_and_barrier = _fast_teardown

B, V = 2048, 50257
N_CORES = 8
ROWS_PER_CORE = B // N_CORES  # 256
P = 128
BLOCKS = ROWS_PER_CORE // P  # 2
F = 3072
# Block 0: plain layout — its end-of-block compute hides under block
# 1's stream.  Block 1 tapers its final chunks: with Scalar doing
# ACT(280ns + 0.85ns/elem) + accumulator-read(278ns) per chunk and DVE
# doing MAX8(170ns + 1.04ns/elem), chunk k+1 must satisfy
#   w_{k+1} >= max(0.636*w_k + 418, 0.778*w_k + 127)
# (transfer pace 1.336 ns/elem) or engine lateness cascades into the
# post-last-byte chain (the old 3072 -> 1201 transition spilled ~1.6us).
CHUNKS0 = [F] * 16 + [1105]  # 49152 + 1105
CHUNKS1 = [2848] + [F] * 12 + [2517, 2085, 1749, 1530, 1391, 1273]
assert sum(CHUNKS0) == V and sum(CHUNKS1) == V
N_RAMP = 6  # block-0 chunks issued upfront on 3 descgen streams

f32 = mybir.dt.float32
Ln = mybir.ActivationFunctionType.Ln
AX = mybir.AxisListType.X

_cache: dict = {}


def _make_nc() -> bass.Bass:
    """Construct Bacc with the Bass preamble slimmed: skip the 4 const-tile
    memsets and the trailing all-engine barrier.  Nothing in this kernel
    reads the const tiles (all activation biases are explicit APs; Copy
    keeps a float bias), and the barrier only ordered those memsets
    against the body — removing both lets the first load descgen start
    ~0.7us earlier."""
    ms_owner = next(
        k for k in bass.BassGpSimd.__mro__ if "memset" in vars(k)
    )
    orig_ms = ms_owner.memset
    orig_aeb = bass.Bass.all_engine_barrier
    ms_owner.memset = lambda self, ap, c: None
    bass.Bass.all_engine_barrier = lambda self, **kw: None
    try:
        nc = bacc.Bacc("TRN2", target_bir_lowering=False)
    finally:
        ms_owner.memset = orig_ms
        bass.Bass.all_engine_barrier = orig_aeb
    return nc


def _offsets(sizes):
    off, out = 0, []
    for sz in sizes:
        out.append((off, sz))
        off += sz
    return out


def _build(top_c: int) -> bass.Bass:
    nc = _make_nc()
    x = nc.dram_tensor("x", [ROWS_PER_CORE, V], f32, kind="ExternalInput")
    # out[0, :C] = per-column row-sums of fin (see column layout below);
    # cols C..127 are zero padding so the store is one 512B line-rate
    # descriptor (a [128,1] per-row store is 128 4-byte RMW writes whose
    # receipts took ~8us under partner-core HBM pressure).
    out = nc.dram_tensor("out", [1, P], f32, kind="ExternalOutput")

    # fin column layout (C = 4 + 2*top_c):
    #   0            : block0 sum of head s_parts            [+]
    #   1            : block0 last-chunk S (scalar accum)    [+]
    #   2 .. 1+c     : block0 lnm values ln(1-m_i)           [-]
    #   2+c          : block1 sum of head s_parts            [+]
    #   3+c          : block1 last-chunk S (scalar accum)    [+]
    #   4+c .. 3+2c  : block1 lnm values                     [-]
    c = top_c
    COL_B0_HEADS = 0
    COL_B0_LAST = 1
    COL_LNM0 = 2
    COL_B1_HEADS = 2 + c
    COL_B1_LAST = 3 + c
    COL_LNM1 = 4 + c
    C = 4 + 2 * c

    chunks0 = _offsets(CHUNKS0)
    chunks1 = _offsets(CHUNKS1)

    with tile.TileContext(nc) as tc:
        with (
            tc.tile_pool(name="xp", bufs=10) as xp,
            tc.tile_pool(name="xsp", bufs=1) as xsp,
            tc.tile_pool(name="yp", bufs=1) as yp,
            tc.tile_pool(name="st", bufs=1) as st,
            tc.tile_pool(name="pp", bufs=1, space=bass.MemorySpace.PSUM) as pp,
        ):
            # DVE-initialized bias tile: keeps the activation-bias const off
            # the Pool-engine prologue, which delays the first load descgen.
            bias_t = st.tile([P, 1], f32, tag="bias_t")
            ostage = st.tile([1, P], f32, tag="ostage")
            fin = st.tile([P, C], f32, tag="fin")
            yt = yp.tile([P, F], f32, tag="yt")

            # ---- block 0 ramp: first N_RAMP loads fan out over three
            # descgen streams (sync/scalar HWDGE + gpsimd SWDGE).  Emitted
            # before any Scalar compute so the scalar-ring descgens are
            # never head-of-line blocked behind a data-gated ACT.
            ramp_engines = (nc.sync, nc.scalar, nc.gpsimd)
            ramp_tiles = []
            for ci in range(N_RAMP):
                coff, sz = chunks0[ci]
                xt = xp.tile([P, sz], f32, tag="xt")
                ramp_engines[ci % 3].dma_start(
                    out=xt[:], in_=x[0:P, coff : coff + sz]
                )
                ramp_tiles.append(xt)
            nc.vector.memset(bias_t[:], 1.0)
            nc.vector.memset(ostage[:], 0.0)

            # ---- block 0 ----
            rows0 = slice(0, P)
            nhead0 = len(chunks0) - 1
            s_parts0 = st.tile([P, nhead0], f32, tag="s_parts0")
            top8s0 = st.tile([P, 8 * nhead0], f32, tag="top8s0")
            top16_0 = st.tile([P, 16], f32, tag="top16_0")
            for ci, (coff, sz) in enumerate(chunks0[:-1]):
                if ci < N_RAMP:
                    xt = ramp_tiles[ci]
                else:
                    xt = xp.tile([P, sz], f32, tag="xt")
                    nc.gpsimd.dma_start(
                        out=xt[:], in_=x[rows0, coff : coff + sz]
                    )
                nc.scalar.activation(
                    yt[:, :sz],
                    xt[:],
                    Ln,
                    bias=bias_t[:, 0:1],
                    scale=-1.0,
                    accum_out=s_parts0[:, ci : ci + 1],
                )
                nc.vector.max(top8s0[:, 8 * ci : 8 * (ci + 1)], xt[:])
            # pre-merge + pre-reduce of the streamed chunks — issued
            # before the last chunk so they run while it is in flight
            nc.vector.max(top16_0[:, 0:8], top8s0[:])
            nc.vector.reduce_sum(
                fin[:, COL_B0_HEADS : COL_B0_HEADS + 1], s_parts0[:], axis=AX
            )
            # last chunk of block 0
            loff, lsz = chunks0[-1]
            xr0 = xsp.tile([P, lsz], f32, tag="xl0")
            nc.gpsimd.dma_start(out=xr0[:], in_=x[rows0, loff : loff + lsz])
            nc.scalar.activation(
                yt[:, :lsz],
                xr0[:],
                Ln,
                bias=bias_t[:, 0:1],
                scale=-1.0,
                accum_out=fin[:, COL_B0_LAST : COL_B0_LAST + 1],
            )
            nc.vector.max(top16_0[:, 8:16], xr0[:])
            m8f0 = st.tile([P, 8], f32, tag="m8f0")
            nc.vector.max(m8f0[:], top16_0[:])
            # block-0 T terms -> fin columns (summed by the final matmul)
            nc.scalar.activation(
                fin[:, COL_LNM0 : COL_LNM0 + c],
                m8f0[:, :c],
                Ln,
                bias=bias_t[:, 0:1],
                scale=-1.0,
            )

            # ---- block 1: head chunks then the tapered tail; identical
            # per-chunk structure throughout (scalar accum + read).
            rows1 = slice(P, 2 * P)
            nh1 = len(chunks1) - 1
            s_parts1 = st.tile([P, nh1], f32, tag="s_parts1")
            top8s1 = st.tile([P, 8 * nh1], f32, tag="top8s1")
            top16_1 = st.tile([P, 16], f32, tag="top16_1")
            for ci, (coff, sz) in enumerate(chunks1[:-1]):
                if sz == F:
                    xt = xp.tile([P, sz], f32, tag="xt")
                else:
                    xt = xsp.tile([P, sz], f32, tag=f"x1_{ci}_{sz}")
                nc.gpsimd.dma_start(out=xt[:], in_=x[rows1, coff : coff + sz])
                nc.scalar.activation(
                    yt[:, :sz],
                    xt[:],
                    Ln,
                    bias=bias_t[:, 0:1],
                    scale=-1.0,
                    accum_out=s_parts1[:, ci : ci + 1],
                )
                nc.vector.max(top8s1[:, 8 * ci : 8 * (ci + 1)], xt[:])
            # pre-merge + pre-reduce, issued before the last chunk so they
            # run while it is in flight
            nc.vector.max(top16_1[:, 0:8], top8s1[:])
            nc.vector.reduce_sum(
                fin[:, COL_B1_HEADS : COL_B1_HEADS + 1], s_parts1[:], axis=AX
            )
            # last chunk — the only compute after its bytes land
            loff, lsz = chunks1[-1]
            xr1 = xsp.tile([P, lsz], f32, tag="xl1")
            nc.gpsimd.dma_start(out=xr1[:], in_=x[rows1, loff : loff + lsz])
            nc.scalar.activation(
                yt[:, :lsz],
                xr1[:],
                Ln,
                bias=bias_t[:, 0:1],
                scale=-1.0,
                accum_out=fin[:, COL_B1_LAST : COL_B1_LAST + 1],
            )
            nc.vector.max(top16_1[:, 8:16], xr1[:])
            m8f1 = st.tile([P, 8], f32, tag="m8f1")
            nc.vector.max(m8f1[:], top16_1[:])
            nc.scalar.activation(
                fin[:, COL_LNM1 : COL_LNM1 + c],
                m8f1[:, :c],
                Ln,
                bias=bias_t[:, 0:1],
                scale=-1.0,
            )

            # Cross-partition row-sum on the (otherwise idle) PE engine:
            # ones.T[1,128] @ fin[128,C] -> [1,C] in PSUM.  This both does
            # the final per-column reductions and consolidates the result
            # into one partition, so the store is a single contiguous
            # 512B write instead of 128 4-byte RMW writes.
            psum = pp.tile([1, C], f32, tag="ps")
            nc.tensor.matmul(psum[:], bias_t[:], fin[:])
            # PSUM->SBUF copy and store descgen both on Scalar: it is free
            # right after lnm1, so no cross-engine hop before descgen.
            nc.scalar.copy(ostage[:, 0:C], psum[:])
            nc.scalar.dma_start(out=out[:], in_=ostage[:])
    nc.compile()
    return nc


def _get(top_c: int) -> bass.Bass:
    if top_c not in _cache:
        _cache[top_c] = _build(top_c)
    return _cache[top_c]


def _signs(top_c: int) -> np.ndarray:
    c = top_c
    sign = np.zeros(P, dtype=np.float64)
    sign[[0, 1, 2 + c, 3 + c]] = 1.0
    sign[2 : 2 + c] = -1.0
    sign[4 + c : 4 + 2 * c] = -1.0
    return sign


def _run(output: np.ndarray, top_c: int, **spmd_kwargs):
    assert 1 <= top_c <= 8, f"kernel supports top_c in [1,8], got {top_c}"
    x = np.ascontiguousarray(np.asarray(output, dtype=np.float32))
    assert x.shape == (B, V), x.shape
    nc = _get(top_c)
    in_maps = [
        {"x": x[i * ROWS_PER_CORE : (i + 1) * ROWS_PER_CORE]} for i in range(N_CORES)
    ]
    res = run_bass_kernel_spmd(nc, in_maps, list(range(N_CORES)), **spmd_kwargs)
    sign = _signs(top_c)
    total = 0.0
    for r in res.results:
        total += float(r["out"].reshape(-1).astype(np.float64) @ sign)
    total = -total / V
    return np.float32(total), res


def kernel(top_c, output) -> np.ndarray:
    val, _ = _run(output, int(top_c))
    return np.array(val, dtype=np.float32)


# revision 11
# speedup vs baseline: 1.0215x; 1.0215x over previous
"""Trainium2 Bass kernel for nn_FLossNoSoftMax (topk_masking).

Computes  -sum_b mean_v[(1-mask)*log(1-x)]  where mask marks the top-c
entries per row of x [2048, 50257] f32.

Math: per row  loss_b = (S_b - T_b)/V  with
  S_b = sum_v log(1-x[b,v])
  T_b = sum over the c largest values m of log(1-m)   (multiset, tie-exact)
result = -sum_b loss_b.

Device kernel (per core, 256 rows = 2 blocks of 128 partitions): stream
row-chunks via SWDGE (gpsimd) loads — the gpsimd ring sustains the
contended HBM rate (~365-390 GB/s with all 8 cores streaming)
gaplessly.  Scalar engine computes Ln(1-x) (fused per-partition
accumulation for the head chunks); vector engine computes per-chunk
top-8 values (InstMax); chunk top-8s merge with InstMax cascades giving
the exact global top-8 multiset per row, whose first c entries yield
the T terms.

Critical-path design (v2 — tail/teardown rework):
- Measured-time anatomy: the NEFF wrapper's epilogue flood (each engine
  serially clearing its ~51-semaphore slice of S[3..255]) is ~6.9us of
  FIXED cost that starts once the slowest engine reaches the wrapper
  barrier, and the reported exec window runs from the first "useful"
  instruction to the very end of that flood.  So every ns shaved off
  the post-stream critical chain (tail compute -> store -> teardown
  arrival) is a ns off the metric.
- The opening loads fan out over THREE descgen streams (sync + scalar
  HWDGE rings, gpsimd SWDGE) to build ring depth fast; the bulk stays
  on gpsimd.  The ramp loads are emitted before any Scalar compute so
  descgen is never head-of-line blocked behind a data-gated ACT.
- Bass's __init__ preamble (4 const-tile memsets + an all-engine
  barrier) is patched out during construction: nothing in this kernel
  reads the const tiles, and the barrier delayed the first load descgen
  by ~0.7us.
- Block 1's stream ends with six shrinking chunks (832..625) sized so
  Scalar/DVE keep pace with the stream's tail; the per-chunk row-sums
  of the five middle tail chunks move to the otherwise-idle Pool
  (gpsimd) engine as tensor_reduce, so Scalar carries no accumulator
  reads there and its post-last-byte chain is just ACT(625)+read+lnm.
- The top-c log terms are NOT accumulated on Scalar: lnm values land in
  fin columns and the final ones.T @ fin matmul (idle PE engine) sums
  them; the host applies per-column signs.  This kills the last
  accumulator-read on the critical chain and consolidates the output
  into one partition so the store is a single 512B line-rate write.
- The final PSUM->SBUF copy and the output store descgen run on the
  SCALAR engine (free right after lnm; the store rides qActDynamicHW
  behind only the two early ramp loads), avoiding a cross-engine hop
  before descgen.
- Teardown (see _fast_teardown): no all-engine barriers of our own;
  GpSimd alone waits for everything (incl. the final store receipt),
  resets DGE state and releases DVE; Sync/Scalar/PE run ahead into the
  NEFF wrapper epilogue.

Output: per-column row-sums in out[0, :C]; host applies signs and the
final -sum/V in float64.
Sharding: data-parallel over the batch dim, 256 rows per core on 8 cores.
"""

import sys

sys.path.insert(0, "/opt/trn_rl_repo")

import numpy as np

from concourse import bacc, bass, mybir, tile
from concourse.bass_utils import run_bass_kernel_spmd
from concourse.vector_clock import ScopedClock


def _ensure_axon_hooks():
    """The agent image lacks antenv.axon_hooks; run_bass_kernel_spmd imports
    it when tracing is requested (e.g. BASS_TRACE=1). Provide the module and
    wire the ctypes NTFF hook so tracing works instead of crashing."""
    try:
        import antenv.axon_hooks  # noqa: F401

        return
    except ImportError:
        pass
    import types

    try:
        import antenv
    except ImportError:
        return
    mod = types.ModuleType("antenv.axon_hooks")
    store = {"h": None}
    mod.set_axon_ntff_profile_hook = lambda h: store.__setitem__("h", h)
    mod.get_axon_ntff_profile_hook = lambda: store.get("h")
    sys.modules["antenv.axon_hooks"] = mod
    antenv.axon_hooks = mod
    try:
        from trn_agent_boot.trn_boot import _ntff_profile_via_ctypes

        mod.set_axon_ntff_profile_hook(
            _ntff_profile_via_ctypes("/opt/axon/libaxon_pjrt.so")
        )
        from concourse import bass_utils as _bu

        _bu.upload_artifacts = lambda d: "local://" + d
    except Exception:
        pass


_ensure_axon_hooks()


def _fast_teardown(self, tick_clock, wait_clock):
    # Replaces Tile's stock drain + 2x all-engine-barrier tail.  The NEFF
    # wrapper's epilogue (an all-engine barrier, then each engine serially
    # clearing a ~50-semaphore slice of S[3..255], then a final barrier)
    # starts once the slowest engine exits our code — so the teardown here
    # avoids gating fast engines on slow ones.  GpSimd alone waits for all
    # outstanding body work (including the final store's HBM write
    # receipt), resets DGE state + clears the tile sems, then releases
    # Vector; Sync/Scalar/PE proceed straight to the wrapper barrier.
    nc = self.nc
    gp = nc.gpsimd.engine
    # Cheap pipeline drains on everything but GpSimd (a GpSimd drain is a
    # dge_drain, which is expensive).
    for eng_type, eng in nc.engines.items():
        if eng_type == gp:
            continue
        d = mybir.InstDrain(
            name=nc.get_next_instruction_name(), ins=[], outs=[],
            bass_is_fusable=False,
        )
        d.engine = eng_type
        eng.add_instruction(d)

    popped = nc._tile_sem_poison_stack.pop()
    assert popped is self._sem_poison

    rel = nc.alloc_semaphore("teardown_release")

    # GpSimd: wait for all outstanding body work (the attached sem waits
    # include every DMA's completion), then reset + clear the tile sems.
    sems = list(self.sems.allocated().values())
    sem_nums = [
        s.num if isinstance(s, bass.SemaphoreHandle) else s for s in sems
    ]
    first = True
    for sem_range in bass.compact_to_ranges(sem_nums):
        assert nc._state.free_isdisjoint(sem_range)
        r = nc.gpsimd.dma_reset(sem_range)
        if first:
            wait_clock.add_sem_waits(
                r.ins, ScopedClock({None: tick_clock.global_clock})
            )
            first = False
        nc.gpsimd.sem_clear(sem_range)
    nc._state.prepend_free_semaphores(sem_nums)
    for poison_set in nc._tile_sem_poison_stack:
        poison_set.update(sem_nums)

    nc.gpsimd.sem_inc(rel, 1)
    nc.vector.wait_ge(rel, 1)
    # Explicit clear so a second NEFF execution starts from 0 even if the
    # wrapper flood's coverage of `rel` shifts.
    nc.vector.sem_clear(range(rel.num, rel.num + 1))


tile.TileContext._drain_and_barrier = _fast_teardown

B, V = 2048, 50257
N_CORES = 8
ROWS_PER_CORE = B // N_CORES  # 256
P = 128
BLOCKS = ROWS_PER_CORE // P  # 2
F = 3072
# Block 0: plain layout — its end-of-block compute hides under block
# 1's stream.  Block 1 tapers its final chunks: with Scalar doing
# ACT(280ns + 0.85ns/elem) + accumulator-read(278ns) per chunk and DVE
# doing MAX8(170ns + 1.04ns/elem), chunk k+1 must satisfy
#   w_{k+1} >= max(0.636*w_k + 418, 0.778*w_k + 127)
# (transfer pace 1.336 ns/elem) or engine lateness cascades into the
# post-last-byte chain (the old 3072 -> 1201 transition spilled ~1.6us).
CHUNKS0 = [F] * 16 + [1105]  # 49152 + 1105
CHUNKS1 = [2848] + [F] * 12 + [2517, 2085, 1749, 1530, 1391, 1273]
assert sum(CHUNKS0) == V and sum(CHUNKS1) == V
N_RAMP = 8  # block-0 chunks issued upfront on 2 descgen streams

f32 = mybir.dt.float32
Ln = mybir.ActivationFunctionType.Ln
AX = mybir.AxisListType.X

_cache: dict = {}


def _make_nc() -> bass.Bass:
    """Construct Bacc with the Bass preamble slimmed: skip the 4 const-tile
    memsets and the trailing all-engine barrier.  Nothing in this kernel
    reads the const tiles (all activation biases are explicit APs; Copy
    keeps a float bias), and the barrier only ordered those memsets
    against the body — removing both lets the first load descgen start
    ~0.7us earlier."""
    ms_owner = next(
        k for k in bass.BassGpSimd.__mro__ if "memset" in vars(k)
    )
    orig_ms = ms_owner.memset
    orig_aeb = bass.Bass.all_engine_barrier
    ms_owner.memset = lambda self, ap, c: None
    bass.Bass.all_engine_barrier = lambda self, **kw: None
    try:
        nc = bacc.Bacc("TRN2", target_bir_lowering=False)
    finally:
        ms_owner.memset = orig_ms
        bass.Bass.all_engine_barrier = orig_aeb
    return nc


def _offsets(sizes):
    off, out = 0, []
    for sz in sizes:
        out.append((off, sz))
        off += sz
    return out


def _build(top_c: int) -> bass.Bass:
    nc = _make_nc()
    x = nc.dram_tensor("x", [ROWS_PER_CORE, V], f32, kind="ExternalInput")
    # out[0, :C] = per-column row-sums of fin (see column layout below);
    # cols C..127 are zero padding so the store is one 512B line-rate
    # descriptor (a [128,1] per-row store is 128 4-byte RMW writes whose
    # receipts took ~8us under partner-core HBM pressure).
    out = nc.dram_tensor("out", [1, P], f32, kind="ExternalOutput")

    # fin column layout (C = 4 + 2*top_c):
    #   0            : block0 sum of head s_parts            [+]
    #   1            : block0 last-chunk S (scalar accum)    [+]
    #   2 .. 1+c     : block0 lnm values ln(1-m_i)           [-]
    #   2+c          : block1 sum of head s_parts            [+]
    #   3+c          : block1 last-chunk S (scalar accum)    [+]
    #   4+c .. 3+2c  : block1 lnm values                     [-]
    c = top_c
    COL_B0_HEADS = 0
    COL_B0_LAST = 1
    COL_LNM0 = 2
    COL_B1_HEADS = 2 + c
    COL_B1_LAST = 3 + c
    COL_LNM1 = 4 + c
    C = 4 + 2 * c

    chunks0 = _offsets(CHUNKS0)
    chunks1 = _offsets(CHUNKS1)

    with tile.TileContext(nc) as tc:
        with (
            tc.tile_pool(name="xp", bufs=10) as xp,
            tc.tile_pool(name="xsp", bufs=1) as xsp,
            tc.tile_pool(name="yp", bufs=1) as yp,
            tc.tile_pool(name="st", bufs=1) as st,
            tc.tile_pool(name="pp", bufs=1, space=bass.MemorySpace.PSUM) as pp,
        ):
            # DVE-initialized bias tile: keeps the activation-bias const off
            # the Pool-engine prologue, which delays the first load descgen.
            bias_t = st.tile([P, 1], f32, tag="bias_t")
            ostage = st.tile([1, P], f32, tag="ostage")
            fin = st.tile([P, C], f32, tag="fin")
            yt = yp.tile([P, F], f32, tag="yt")

            # ---- block 0 ramp: the first N_RAMP loads alternate between
            # the HWDGE (sync) and SWDGE (gpsimd) rings: HWDGE has the
            # faster first-byte path, and two descriptor-generation streams
            # build ring depth twice as fast through the ramp.  Do NOT use
            # the scalar HWDGE ring for loads: a third active dynamic queue
            # during the stream adds ~43ns to every packet on SDMA engine
            # 79 (which hosts the dynamic queues' processing), skewing it
            # ~12us behind its 15 siblings.
            ramp_tiles = []
            for ci in range(N_RAMP):
                coff, sz = chunks0[ci]
                xt = xp.tile([P, sz], f32, tag="xt")
                eng = nc.sync if ci % 2 == 0 else nc.gpsimd
                eng.dma_start(out=xt[:], in_=x[0:P, coff : coff + sz])
                ramp_tiles.append(xt)
            nc.vector.memset(bias_t[:], 1.0)
            nc.vector.memset(ostage[:], 0.0)

            # ---- block 0 ----
            rows0 = slice(0, P)
            nhead0 = len(chunks0) - 1
            s_parts0 = st.tile([P, nhead0], f32, tag="s_parts0")
            top8s0 = st.tile([P, 8 * nhead0], f32, tag="top8s0")
            top16_0 = st.tile([P, 16], f32, tag="top16_0")
            for ci, (coff, sz) in enumerate(chunks0[:-1]):
                if ci < N_RAMP:
                    xt = ramp_tiles[ci]
                else:
                    xt = xp.tile([P, sz], f32, tag="xt")
                    nc.gpsimd.dma_start(
                        out=xt[:], in_=x[rows0, coff : coff + sz]
                    )
                nc.scalar.activation(
                    yt[:, :sz],
                    xt[:],
                    Ln,
                    bias=bias_t[:, 0:1],
                    scale=-1.0,
                    accum_out=s_parts0[:, ci : ci + 1],
                )
                nc.vector.max(top8s0[:, 8 * ci : 8 * (ci + 1)], xt[:])
            # pre-merge + pre-reduce of the streamed chunks — issued
            # before the last chunk so they run while it is in flight
            nc.vector.max(top16_0[:, 0:8], top8s0[:])
            nc.vector.reduce_sum(
                fin[:, COL_B0_HEADS : COL_B0_HEADS + 1], s_parts0[:], axis=AX
            )
            # last chunk of block 0
            loff, lsz = chunks0[-1]
            xr0 = xsp.tile([P, lsz], f32, tag="xl0")
            nc.gpsimd.dma_start(out=xr0[:], in_=x[rows0, loff : loff + lsz])
            nc.scalar.activation(
                yt[:, :lsz],
                xr0[:],
                Ln,
                bias=bias_t[:, 0:1],
                scale=-1.0,
                accum_out=fin[:, COL_B0_LAST : COL_B0_LAST + 1],
            )
            nc.vector.max(top16_0[:, 8:16], xr0[:])
            m8f0 = st.tile([P, 8], f32, tag="m8f0")
            nc.vector.max(m8f0[:], top16_0[:])
            # block-0 T terms -> fin columns (summed by the final matmul)
            nc.scalar.activation(
                fin[:, COL_LNM0 : COL_LNM0 + c],
                m8f0[:, :c],
                Ln,
                bias=bias_t[:, 0:1],
                scale=-1.0,
            )

            # ---- block 1: head chunks then the tapered tail; identical
            # per-chunk structure throughout (scalar accum + read).
            rows1 = slice(P, 2 * P)
            nh1 = len(chunks1) - 1
            s_parts1 = st.tile([P, nh1], f32, tag="s_parts1")
            top8s1 = st.tile([P, 8 * nh1], f32, tag="top8s1")
            top16_1 = st.tile([P, 16], f32, tag="top16_1")
            for ci, (coff, sz) in enumerate(chunks1[:-1]):
                if sz == F:
                    xt = xp.tile([P, sz], f32, tag="xt")
                else:
                    xt = xsp.tile([P, sz], f32, tag=f"x1_{ci}_{sz}")
                nc.gpsimd.dma_start(out=xt[:], in_=x[rows1, coff : coff + sz])
                nc.scalar.activation(
                    yt[:, :sz],
                    xt[:],
                    Ln,
                    bias=bias_t[:, 0:1],
                    scale=-1.0,
                    accum_out=s_parts1[:, ci : ci + 1],
                )
                nc.vector.max(top8s1[:, 8 * ci : 8 * (ci + 1)], xt[:])
            # pre-merge + pre-reduce, issued before the last chunk so they
            # run while it is in flight
            nc.vector.max(top16_1[:, 0:8], top8s1[:])
            nc.vector.reduce_sum(
                fin[:, COL_B1_HEADS : COL_B1_HEADS + 1], s_parts1[:], axis=AX
            )
            # last chunk — the only compute after its bytes land
            loff, lsz = chunks1[-1]
            xr1 = xsp.tile([P, lsz], f32, tag="xl1")
            nc.gpsimd.dma_start(out=xr1[:], in_=x[rows1, loff : loff + lsz])
            nc.scalar.activation(
                yt[:, :lsz],
                xr1[:],
                Ln,
                bias=bias_t[:, 0:1],
                scale=-1.0,
                accum_out=fin[:, COL_B1_LAST : COL_B1_LAST + 1],
            )
            nc.vector.max(top16_1[:, 8:16], xr1[:])
            m8f1 = st.tile([P, 8], f32, tag="m8f1")
            nc.vector.max(m8f1[:], top16_1[:])
            nc.scalar.activation(
                fin[:, COL_LNM1 : COL_LNM1 + c],
                m8f1[:, :c],
                Ln,
                bias=bias_t[:, 0:1],
                scale=-1.0,
            )

            # Cross-partition row-sum on the (otherwise idle) PE engine:
            # ones.T[1,128] @ fin[128,C] -> [1,C] in PSUM.  This both does
            # the final per-column reductions and consolidates the result
            # into one partition, so the store is a single contiguous
            # 512B write instead of 128 4-byte RMW writes.
            psum = pp.tile([1, C], f32, tag="ps")
            nc.tensor.matmul(psum[:], bias_t[:], fin[:])
            # PSUM->SBUF copy and store descgen both on Scalar: it is free
            # right after lnm1, so no cross-engine hop before descgen.
            nc.scalar.copy(ostage[:, 0:C], psum[:])
            nc.scalar.dma_start(out=out[:], in_=ostage[:])
    nc.compile()
    return nc


def _get(top_c: int) -> bass.Bass:
    if top_c not in _cache:
        _cache[top_c] = _build(top_c)
    return _cache[top_c]


def _signs(top_c: int) -> np.ndarray:
    c = top_c
    sign = np.zeros(P, dtype=np.float64)
    sign[[0, 1, 2 + c, 3 + c]] = 1.0
    sign[2 : 2 + c] = -1.0
    sign[4 + c : 4 + 2 * c] = -1.0
    return sign


def _run(output: np.ndarray, top_c: int, **spmd_kwargs):
    assert 1 <= top_c <= 8, f"kernel supports top_c in [1,8], got {top_c}"
    x = np.ascontiguousarray(np.asarray(output, dtype=np.float32))
    assert x.shape == (B, V), x.shape
    nc = _get(top_c)
    in_maps = [
        {"x": x[i * ROWS_PER_CORE : (i + 1) * ROWS_PER_CORE]} for i in range(N_CORES)
    ]
    res = run_bass_kernel_spmd(nc, in_maps, list(range(N_CORES)), **spmd_kwargs)
    sign = _signs(top_c)
    total = 0.0
    for r in res.results:
        total += float(r["out"].reshape(-1).astype(np.float64) @ sign)
    total = -total / V
    return np.float32(total), res


def kernel(top_c, output) -> np.ndarray:
    val, _ = _run(output, int(top_c))
    return np.array(val, dtype=np.float32)
